# revision 62
# baseline (speedup 1.0000x reference)
"""CQAttention Trainium2 kernel — data-parallel over batch across 8 NeuronCores.

Problem shapes (hardcoded): B=32, H=256, Lc=1024, Lq=256.
Each core processes B/8 = 4 batches.

Math (per batch, with all-ones masks — guaranteed by the problem spec):
  Ct = C^T [Lc,H], Qt = Q^T [Lq,H]
  S[l,m] = r[l] + q[m] + (Ct[l]*w3)@Qt[m]   (r = Ct@w1, q = Qt@w2)
  Z = exp(S) serves BOTH softmaxes:
    S_row = Z / rowsum(Z),  S_col = Z / colsum(Z)
  A  = S_row @ Qt
  Bv = S_row @ (S_col^T @ Ct)      (factored: avoids the Lc x Lc product)
  out = relu([Ct, A, Ct*A, Ct*Bv] @ W_res^T + b_res)^T  -> [H, Lc]

Implementation notes (fully on-chip; no DRAM bounces, no DMA on the
critical path):
  - Logit matmuls run in bf16 (PE 1 cyc/col; accumulation is fp32 in
    PSUM). Everything downstream of exp is bf16. rel err ~4e-3 vs the
    2e-2 gate.
  - Z is computed in both [l,m] and [m,l] layouts (each is needed as a
    contraction operand with l resp. m on partitions); exp's accum_out
    yields the row/col softmax sums for free.
  - The r[l]/q[m] bias columns for exp are produced on-chip: r/q rows
    come from w1.C / w2.Q matmuls, then ten tiny N=1 matmuls
    (row_chunk^T @ [1.0]) flip them into one [128,10] PSUM column set.
  - 1/rho is broadcast on-chip: rho column [128,8] -> PE transpose ->
    [8,128] rows -> eight K=8 selector matmuls -> ri_bc [128,Lc]. The
    A/Bv PSUM drains multiply by ri_bc directly (the scaled attention
    matrix never materializes), so no DMA round-trip gates the batch.
  - PE transposes drain 4-at-a-time through [128,512] PSUM tiles into
    wide contiguous tiles (CtT_all/QT_all/WT_all).
  - Engine split: ACT = exp/relu/T-scale; Vector = PSUM drains, casts,
    products; GpSimd (no PSUM port) = SBUF-only affine operands.
    Never write a tile in sub-ranges from a compute engine and read it
    from PE LDWEIGHTS — Tile misses that dependency (observed miscompute).
  - Two HWDGE queues: SP carries the per-batch C/Q loads (prefetched two
    batches ahead) + output stores; the Activation queue carries the
    one-time loads (W_res, b_col) so they never delay batch-0 inputs;
    w loads as a single combined DMA; sel8 is built by affine_select.
  - Emission is software-pipelined: loads(b+2), frontend(b+1), backend(b).
"""

import numpy as np

_CACHE = {}

B_FULL = 32
N_CORES = 8
BB = B_FULL // N_CORES  # batches per core = 4
H = 256
LC = 1024
LQ = 256


def _build(reps: int = 1):
    from contextlib import ExitStack

    import concourse.bass as bass
    import concourse.tile as tile
    from concourse import bacc, mybir
    from concourse.masks import make_identity

    f32 = mybir.dt.float32
    f32r = mybir.dt.float32r
    bf16 = mybir.dt.bfloat16
    fp8 = mybir.dt.float8e4
    AF = mybir.ActivationFunctionType
    OP = mybir.AluOpType

    nc = bacc.Bacc("TRN2", target_bir_lowering=False, debug=False)

    def mm(out, lhsT, rhs, start, stop):
        nc.tensor.matmul(
            out,
            lhsT=lhsT.bitcast(f32r),
            rhs=rhs.bitcast(f32r),
            start=start,
            stop=stop,
        )

    def mmb(out, lhsT, rhs, start, stop):
        nc.tensor.matmul(out, lhsT=lhsT, rhs=rhs, start=start, stop=stop)

    C = nc.dram_tensor("C", [BB, H, LC], f32, kind="ExternalInput")
    Q = nc.dram_tensor("Q", [BB, H, LQ], f32, kind="ExternalInput")
    w = nc.dram_tensor("w", [3 * H], f32, kind="ExternalInput")
    W_res = nc.dram_tensor("W_res", [H, 4 * H], f32, kind="ExternalInput")
    b_res = nc.dram_tensor("b_res", [H], f32, kind="ExternalInput")
    out = nc.dram_tensor("out", [BB, H, LC], f32, kind="ExternalOutput")

    KH = H // 128  # 2 h-chunks
    NLT = LC // 128  # 8 l-tiles
    NMT = LQ // 128  # 2 m-tiles

    with tile.TileContext(nc) as tc:
        with ExitStack() as ctx:
            singles = ctx.enter_context(tc.tile_pool(name="singles", bufs=1))
            sb = ctx.enter_context(tc.tile_pool(name="sb", bufs=2))
            sb1 = ctx.enter_context(tc.tile_pool(name="sb1", bufs=3))
            sbig = ctx.enter_context(tc.tile_pool(name="sbig", bufs=2))
            sbig1 = ctx.enter_context(tc.tile_pool(name="sbig1", bufs=3))
            ps_tr = ctx.enter_context(
                tc.tile_pool(name="ps_tr", bufs=2, space="PSUM")
            )
            ps_z = ctx.enter_context(
                tc.tile_pool(name="ps_z", bufs=2, space="PSUM")
            )
            ps_big = ctx.enter_context(
                tc.tile_pool(name="ps_big", bufs=2, space="PSUM")
            )

            # ---- one-time constants ----
            identity_bf = singles.tile([128, 128], bf16)
            make_identity(nc, identity_bf)


            # w = [w1|w2|w3] is contiguous in DRAM: one DMA loads all three
            # column sets ([128, 6]; element 128*i + p -> (p, i)), saving two
            # ~0.9us descriptor slots on the SP queue ahead of the C0 load.
            w_cols = singles.tile([128, 3 * KH], f32r)
            nc.sync.dma_start(
                out=w_cols,
                in_=w.ap()
                .rearrange("(i p) -> p i", i=3 * KH, p=128)
                .bitcast(f32r),
            )
            w1_col = w_cols[:, 0:KH]
            w2_col = w_cols[:, KH : 2 * KH]
            w3_col = w_cols[:, 2 * KH : 3 * KH].bitcast(f32)
            # W_res^T (bf16): WT_all[:, 256*f + 128*j : +128] = W_res[128j:128(j+1), 128f:128(f+1)]^T
            # Two loads only — early transfers compete with the batch-0 C/Q
            # loads for DMA hardware rings, so fewer is better.
            WT_all = singles.tile([128, 4 * H * KH], bf16)
            wn = {}
            for j in range(KH):
                t = singles.tile([128, 4 * H], f32, tag=f"wn{j}")
                nc.scalar.dma_start(out=t, in_=W_res.ap()[128 * j : 128 * (j + 1), :])
                tb = singles.tile([128, 4 * H], bf16, tag=f"wnb{j}")
                nc.vector.tensor_copy(tb, t)
                wn[j] = tb
            b_col = singles.tile([128, KH], f32)
            nc.scalar.dma_start(
                out=b_col, in_=b_res.ap().rearrange("(i p) -> p i", i=KH, p=128)
            )

            # sel8[k, 128*i + q] = (k == i): K=8 selector for broadcasting row i
            # of an [8, 128] tile to all 128 output partitions via one matmul.
            # Built with two affine_selects (keep where pred true, else fill 0)
            # bracketing the band 128p <= f < 128(p+1) — no DMAs, so the ACT
            # queue reaches batch-0's exps immediately.
            sel8 = singles.tile([NLT, NLT * 128], bf16)
            nc.gpsimd.memset(sel8, 1.0)
            nc.gpsimd.affine_select(
                out=sel8,
                in_=sel8,
                compare_op=OP.is_ge,
                fill=0.0,
                base=0,
                pattern=[[1, NLT * 128]],  # iota = f - 128*p
                channel_multiplier=-128,
            )
            nc.gpsimd.affine_select(
                out=sel8,
                in_=sel8,
                compare_op=OP.is_ge,
                fill=0.0,
                base=127,
                pattern=[[-1, NLT * 128]],  # iota = 127 + 128*p - f
                channel_multiplier=128,
            )

            seq = [(f, j) for f in range(8) for j in range(KH)]
            for g in range(4):
                pt = ps_tr.tile([128, 512], bf16, tag="tr")
                for s in range(4):
                    f, j = seq[4 * g + s]
                    nc.tensor.transpose(
                        pt[:, 128 * s : 128 * (s + 1)],
                        wn[j][:, 128 * f : 128 * (f + 1)],
                        identity_bf,
                    )
                nc.vector.tensor_copy(
                    out=WT_all[:, 512 * g : 512 * (g + 1)], in_=pt
                )

            def wt(f, t_i):
                return WT_all[:, 256 * f + 128 * t_i : 256 * f + 128 * (t_i + 1)]

            def loads(b):
                C_nat = []
                Q_nat = []
                for k in range(KH):
                    t = sbig.tile([128, LC], f32r, tag=f"cnat{k}", bufs=3)
                    nc.sync.dma_start(
                        out=t,
                        in_=C.ap()[b, 128 * k : 128 * (k + 1), :].bitcast(f32r),
                    )
                    C_nat.append(t)
                    tq = sb.tile([128, LQ], f32r, tag=f"qnat{k}", bufs=3)
                    nc.sync.dma_start(
                        out=tq,
                        in_=Q.ap()[b, 128 * k : 128 * (k + 1), :].bitcast(f32r),
                    )
                    Q_nat.append(tq)
                return C_nat, Q_nat

            def frontend(b, ld):
                st = {}
                C_nat, Q_nat = ld

                # ---- r_row = w1.C  [1, LC],  q_row = w2.Q  [1, LQ] (bf16) ----
                r_row = sb.tile([1, LC], bf16, tag="rrow")
                for c in range(2):
                    ps_r = ps_tr.tile([1, 512], f32, tag="tr")
                    for k in range(KH):
                        mm(
                            ps_r,
                            w1_col[:, k : k + 1],
                            C_nat[k][:, 512 * c : 512 * (c + 1)],
                            (k == 0),
                            (k == KH - 1),
                        )
                    nc.vector.tensor_copy(
                        out=r_row[:, 512 * c : 512 * (c + 1)], in_=ps_r
                    )
                ps_q = ps_tr.tile([1, LQ], f32, tag="tr")
                for k in range(KH):
                    mm(
                        ps_q,
                        w2_col[:, k : k + 1],
                        Q_nat[k],
                        (k == 0),
                        (k == KH - 1),
                    )
                q_row = sb.tile([1, LQ], bf16, tag="qrow")
                nc.vector.tensor_copy(out=q_row, in_=ps_q)

                # ---- flip r/q rows into per-partition bias columns via ten
                # ---- tiny N=1 matmuls (lhsT^T @ [1.0]), batched in one PSUM ----
                ptr_rq = ps_tr.tile([128, 16], f32, tag="tr")
                for i in range(NLT):
                    mmb(
                        ptr_rq[:, i : i + 1],
                        r_row[:, 128 * i : 128 * (i + 1)],
                        identity_bf[0:1, 0:1],
                        True,
                        True,
                    )
                for j in range(NMT):
                    mmb(
                        ptr_rq[:, NLT + j : NLT + j + 1],
                        q_row[:, 128 * j : 128 * (j + 1)],
                        identity_bf[0:1, 0:1],
                        True,
                        True,
                    )
                rq_col = sb.tile([128, NLT + NMT], f32, tag="rqcol")
                nc.vector.tensor_copy(rq_col, ptr_rq[:, 0 : NLT + NMT])

                # ---- bf16 copies ----
                C_bf = []
                Q_bf = []
                for k in range(KH):
                    cb = sbig.tile([128, LC], bf16, tag=f"cbf{k}")
                    nc.vector.tensor_copy(cb, C_nat[k].bitcast(f32))
                    C_bf.append(cb)
                    qb = sb.tile([128, LQ], bf16, tag=f"qbf{k}")
                    nc.vector.tensor_copy(qb, Q_nat[k].bitcast(f32))
                    Q_bf.append(qb)

                # ---- PE transposes, batched drains ----
                # CtT_all[:, 256*i + 128*k : +128] = C^T l-tile i, h-chunk k
                CtT_all = sb1.tile([128, 2 * H * NLT // 2], bf16, tag="ctt")
                cseq = [(i, k) for i in range(NLT) for k in range(KH)]
                for g in range(4):
                    pt = ps_tr.tile([128, 512], bf16, tag="tr")
                    for s in range(4):
                        i, k = cseq[4 * g + s]
                        nc.tensor.transpose(
                            pt[:, 128 * s : 128 * (s + 1)],
                            C_bf[k][:, 128 * i : 128 * (i + 1)],
                            identity_bf,
                        )
                    nc.vector.tensor_copy(
                        out=CtT_all[:, 512 * g : 512 * (g + 1)], in_=pt
                    )
                QT_all = sb.tile([128, H * NMT], bf16, tag="qt")
                qseq = [(j, k) for j in range(NMT) for k in range(KH)]
                pt = ps_tr.tile([128, 512], bf16, tag="tr")
                for s in range(4):
                    j, k = qseq[s]
                    nc.tensor.transpose(
                        pt[:, 128 * s : 128 * (s + 1)],
                        Q_bf[k][:, 128 * j : 128 * (j + 1)],
                        identity_bf,
                    )
                nc.vector.tensor_copy(out=QT_all, in_=pt)

                # ---- affine-augmented operands (GpSimd: SBUF-only) ----
                # CA = C*w3 + w2 so CA^T@Q = dot + q[m]; QA = Q*w3 + w1 so
                # QA^T@C = dot + r[l]. bf16: the PE runs bf16 at 1 cyc/col
                # vs ~1.1-1.4 for fp32r, and accumulation stays fp32.
                CA = []
                QA = []
                for k in range(KH):
                    t = sbig.tile([128, LC], bf16, tag=f"ca{k}")
                    eng = nc.gpsimd if k == 0 else nc.vector
                    eng.tensor_scalar(
                        out=t,
                        in0=C_nat[k],
                        scalar1=w3_col[:, k : k + 1],
                        scalar2=w2_col[:, k : k + 1].bitcast(f32),
                        op0=OP.mult,
                        op1=OP.add,
                    )
                    CA.append(t)
                    tq = sb.tile([128, LQ], bf16, tag=f"qa{k}")
                    nc.vector.tensor_scalar(
                        out=tq,
                        in0=Q_nat[k],
                        scalar1=w3_col[:, k : k + 1],
                        scalar2=w1_col[:, k : k + 1].bitcast(f32),
                        op0=OP.mult,
                        op1=OP.add,
                    )
                    QA.append(tq)

                st.update(
                    C_nat=C_nat, Q_nat=Q_nat, C_bf=C_bf, Q_bf=Q_bf,
                    CtT_all=CtT_all, QT_all=QT_all, CA=CA, QA=QA,
                    rq_col=rq_col,
                )
                return st

            def backend(b, st):
                C_nat = st["C_nat"]; Q_nat = st["Q_nat"]; C_bf = st["C_bf"]
                Q_bf = st["Q_bf"]; CtT_all = st["CtT_all"]; QT_all = st["QT_all"]
                CA = st["CA"]; QA = st["QA"]; rq_col = st["rq_col"]

                # ---- Z in [l, m] layout + rowsums rho ----
                # S = (C*w3)^T Q + r x 1 + 1 x q ; the rank-1 terms enter the
                # PSUM accumulation directly (fp32r), no exp-bias needed.
                # Two l-tiles share each [128, 512] PSUM tile (the ring
                # slots are 2KB anyway): separate accumulation groups and
                # exps per half, but the 2-slot ring now gives each matmul
                # group four exps of slack instead of two.
                rho_col = sb.tile([128, NLT], f32, tag="rho")
                E_lm = []
                for p in range(NLT // 2):
                    pz = ps_z.tile([128, 2 * LQ], f32, tag="z")
                    for h in range(2):
                        i = 2 * p + h
                        sl = slice(LQ * h, LQ * (h + 1))
                        for k in range(KH):
                            mmb(
                                pz[:, sl],
                                CA[k][:, 128 * i : 128 * (i + 1)],
                                Q_bf[k],
                                (k == 0),
                                (k == KH - 1),
                            )
                        e = sb1.tile([128, LQ], bf16, tag=f"elm{i}", name=f"elm{i}")
                        nc.scalar.activation(
                            out=e,
                            in_=pz[:, sl],
                            func=AF.Exp,
                            bias=rq_col[:, i : i + 1],
                            accum_out=rho_col[:, i : i + 1],
                        )
                        E_lm.append(e)

                # ---- Z in [m, l] layout + colsums kappa ----
                kap_col = sb.tile([128, NMT], f32, tag="kap")
                E_ml = []
                for j in range(NMT):
                    pzt = ps_big.tile([128, LC], f32, tag="big")
                    for c in range(2):
                        sl = slice(512 * c, 512 * (c + 1))
                        for k in range(KH):
                            mmb(
                                pzt[:, sl],
                                QA[k][:, 128 * j : 128 * (j + 1)],
                                C_bf[k][:, sl],
                                (k == 0),
                                (k == KH - 1),
                            )
                    e = sbig1.tile([128, LC], bf16, tag=f"eml{j}")
                    nc.scalar.activation(
                        out=e,
                        in_=pzt,
                        func=AF.Exp,
                        bias=rq_col[:, NLT + j : NLT + j + 1],
                        accum_out=kap_col[:, j : j + 1],
                    )
                    E_ml.append(e)

                # ---- reciprocals ----
                kap_inv = sb.tile([128, NMT], f32, tag="kapi")
                nc.vector.reciprocal(kap_inv, kap_col)

                # ---- T = S_col^T @ Ct   [m, h] ----
                T_nat = []
                for j in range(NMT):
                    pT = ps_z.tile([128, H], f32, tag="z")
                    for i in range(NLT):
                        mmb(
                            pT,
                            E_lm[i][:, 128 * j : 128 * (j + 1)],
                            CtT_all[:, 256 * i : 256 * (i + 1)],
                            (i == 0),
                            (i == NLT - 1),
                        )
                    t = sb1.tile([128, H], bf16, tag=f"tn{j}")
                    nc.scalar.activation(
                        out=t, in_=pT, func=AF.Copy, scale=kap_inv[:, j : j + 1]
                    )
                    T_nat.append(t)

                # ---- 1/rho broadcast: column -> rows -> [128, LC] ----
                rho_inv = sb.tile([128, NLT], f32, tag="rhoi")
                nc.vector.reciprocal(rho_inv, rho_col)
                rho_inv_bf = sb.tile([128, NLT], bf16, tag="rhoib")
                nc.gpsimd.tensor_copy(rho_inv_bf, rho_inv)
                ptr = ps_tr.tile([NLT, 128], bf16, tag="tr")
                nc.tensor.transpose(ptr, rho_inv_bf, identity_bf)
                rho_rows = sb.tile([NLT, 128], bf16, tag="rrows")
                nc.vector.tensor_copy(rho_rows, ptr)
                ri_bc = sbig1.tile([128, LC], bf16, tag="ribc")
                for half in range(2):
                    pri = ps_tr.tile([128, 512], f32, tag="tr")
                    for s in range(4):
                        i = 4 * half + s
                        mmb(
                            pri[:, 128 * s : 128 * (s + 1)],
                            sel8[:, 128 * i : 128 * (i + 1)],
                            rho_rows,
                            True,
                            True,
                        )
                    nc.vector.tensor_copy(
                        out=ri_bc[:, 512 * half : 512 * (half + 1)], in_=pri
                    )

                # ---- A^T and Bv^T  [h, l]: matmuls on unscaled E_ml, the
                # ---- PSUM drain multiplies in 1/rho[l] ----
                A_T = []
                Bv_T = []
                for t_i in range(KH):
                    pA = ps_big.tile([128, LC], f32, tag="big")
                    for k in range(NMT):
                        for c in range(2):
                            sl = slice(512 * c, 512 * (c + 1))
                            mmb(
                                pA[:, sl],
                                QT_all[:, 256 * k + 128 * t_i : 256 * k + 128 * (t_i + 1)],
                                E_ml[k][:, sl],
                                (k == 0),
                                (k == NMT - 1),
                            )
                    a = sbig1.tile([128, LC], bf16, tag=f"at{t_i}")
                    nc.vector.tensor_mul(a, pA, ri_bc)
                    A_T.append(a)
                for t_i in range(KH):
                    pB = ps_big.tile([128, LC], f32, tag="big")
                    for k in range(NMT):
                        for c in range(2):
                            sl = slice(512 * c, 512 * (c + 1))
                            mmb(
                                pB[:, sl],
                                T_nat[k][:, 128 * t_i : 128 * (t_i + 1)],
                                E_ml[k][:, sl],
                                (k == 0),
                                (k == NMT - 1),
                            )
                    bv = sbig1.tile([128, LC], bf16, tag=f"bvt{t_i}")
                    nc.vector.tensor_mul(bv, pB, ri_bc)
                    Bv_T.append(bv)

                # ---- products ----
                CA1 = []
                CB1 = []
                for t_i in range(KH):
                    p1 = sbig1.tile([128, LC], bf16, tag=f"ca1{t_i}")
                    nc.vector.tensor_tensor(
                        out=p1, in0=C_bf[t_i], in1=A_T[t_i], op=OP.mult
                    )
                    CA1.append(p1)
                    p2 = sbig1.tile([128, LC], bf16, tag=f"cb1{t_i}")
                    nc.vector.tensor_tensor(
                        out=p2, in0=C_bf[t_i], in1=Bv_T[t_i], op=OP.mult
                    )
                    CB1.append(p2)

                # ---- final matmul + relu + store ----
                blocks = [
                    C_bf[0],
                    C_bf[1],
                    A_T[0],
                    A_T[1],
                    CA1[0],
                    CA1[1],
                    CB1[0],
                    CB1[1],
                ]
                # c-outer so each 512-chunk drains (relu + store) while the
                # PE accumulates the other chunk.
                for t_i in range(KH):
                    po = ps_big.tile([128, LC], f32, tag="big")
                    for c in range(2):
                        sl = slice(512 * c, 512 * (c + 1))
                        for f in range(8):
                            mmb(
                                po[:, sl],
                                wt(f, t_i),
                                blocks[f][:, sl],
                                (f == 0),
                                (f == 7),
                            )
                        o = sbig.tile([128, 512], f32, tag=f"osb{t_i}{c}")
                        nc.scalar.activation(
                            out=o,
                            in_=po[:, sl],
                            func=AF.Relu,
                            bias=b_col[:, t_i : t_i + 1],
                        )
                        nc.sync.dma_start(
                            out=out.ap()[b, 128 * t_i : 128 * (t_i + 1), sl],
                            in_=o,
                        )

            def body(iv=None):
                ld = {0: loads(0), 1: loads(1)}
                st_prev = None
                for b in range(BB):
                    if b + 2 < BB:
                        ld[b + 2] = loads(b + 2)
                    st = frontend(b, ld.pop(b))
                    if st_prev is not None:
                        backend(b - 1, st_prev)
                    st_prev = st
                backend(BB - 1, st_prev)

            if reps == 1:
                body()
            else:
                with tc.For_i(0, reps, 1) as iv:
                    body(iv)

    nc.compile()
    return nc


def _get_nc(reps: int = 1):
    key = ("nc", reps)
    if key not in _CACHE:
        _CACHE[key] = _build(reps)
    return _CACHE[key]


def kernel(C, Q, cmask, qmask, w, W_res, b_res, _reps: int = 1, _want_res: bool = False,
           _trace: bool = False, _tmpdir: str | None = None):
    from concourse.bass_utils import run_bass_kernel_spmd

    nc = _get_nc(_reps)

    C = np.ascontiguousarray(C, dtype=np.float32)
    Q = np.ascontiguousarray(Q, dtype=np.float32)
    w = np.ascontiguousarray(w, dtype=np.float32)
    W_res = np.ascontiguousarray(W_res, dtype=np.float32)
    b_res = np.ascontiguousarray(b_res, dtype=np.float32)

    in_maps = []
    for i in range(N_CORES):
        sl = slice(i * BB, (i + 1) * BB)
        in_maps.append(
            {"C": C[sl], "Q": Q[sl], "w": w, "W_res": W_res, "b_res": b_res}
        )

    res = run_bass_kernel_spmd(
        nc, in_maps, core_ids=list(range(N_CORES)), trace=_trace, tmpdir=_tmpdir
    )
    out = np.concatenate([res.results[i]["out"] for i in range(N_CORES)], axis=0)
    if _want_res:
        return out, res
    return out


# revision 64
# speedup vs baseline: 1.0010x; 1.0010x over previous
"""CQAttention Trainium2 kernel — data-parallel over batch across 8 NeuronCores.

Problem shapes (hardcoded): B=32, H=256, Lc=1024, Lq=256.
Each core processes B/8 = 4 batches.

Math (per batch, with all-ones masks — guaranteed by the problem spec):
  Ct = C^T [Lc,H], Qt = Q^T [Lq,H]
  S[l,m] = r[l] + q[m] + (Ct[l]*w3)@Qt[m]   (r = Ct@w1, q = Qt@w2)
  Z = exp(S) serves BOTH softmaxes:
    S_row = Z / rowsum(Z),  S_col = Z / colsum(Z)
  A  = S_row @ Qt
  Bv = S_row @ (S_col^T @ Ct)      (factored: avoids the Lc x Lc product)
  out = relu([Ct, A, Ct*A, Ct*Bv] @ W_res^T + b_res)^T  -> [H, Lc]

Implementation notes (fully on-chip; no DRAM bounces, no DMA on the
critical path):
  - Logit matmuls run in bf16 (PE 1 cyc/col; accumulation is fp32 in
    PSUM). Everything downstream of exp is bf16. rel err ~4e-3 vs the
    2e-2 gate.
  - Z is computed in both [l,m] and [m,l] layouts (each is needed as a
    contraction operand with l resp. m on partitions); exp's accum_out
    yields the row/col softmax sums for free.
  - The r[l]/q[m] bias columns for exp are produced on-chip: r/q rows
    come from w1.C / w2.Q matmuls, then ten tiny N=1 matmuls
    (row_chunk^T @ [1.0]) flip them into one [128,10] PSUM column set.
  - 1/rho is broadcast on-chip: rho column [128,8] -> PE transpose ->
    [8,128] rows -> eight K=8 selector matmuls -> ri_bc [128,Lc]. The
    A/Bv PSUM drains multiply by ri_bc directly (the scaled attention
    matrix never materializes), so no DMA round-trip gates the batch.
  - PE transposes drain 4-at-a-time through [128,512] PSUM tiles into
    wide contiguous tiles (CtT_all/QT_all/WT_all).
  - Engine split: ACT = exp/relu/T-scale; Vector = PSUM drains, casts,
    products; GpSimd (no PSUM port) = SBUF-only affine operands.
    Never write a tile in sub-ranges from a compute engine and read it
    from PE LDWEIGHTS — Tile misses that dependency (observed miscompute).
  - Two HWDGE queues: SP carries the per-batch C/Q loads (prefetched two
    batches ahead) + output stores; the Activation queue carries the
    one-time loads (W_res, b_col) so they never delay batch-0 inputs;
    w loads as a single combined DMA; sel8 is built by affine_select.
  - Emission is software-pipelined: loads(b+2), frontend(b+1), backend(b).
"""

import numpy as np

_CACHE = {}

B_FULL = 32
N_CORES = 8
BB = B_FULL // N_CORES  # batches per core = 4
H = 256
LC = 1024
LQ = 256


def _build(reps: int = 1):
    from contextlib import ExitStack

    import concourse.bass as bass
    import concourse.tile as tile
    from concourse import bacc, mybir
    from concourse.masks import make_identity

    f32 = mybir.dt.float32
    f32r = mybir.dt.float32r
    bf16 = mybir.dt.bfloat16
    fp8 = mybir.dt.float8e4
    AF = mybir.ActivationFunctionType
    OP = mybir.AluOpType

    nc = bacc.Bacc("TRN2", target_bir_lowering=False, debug=False)

    def mm(out, lhsT, rhs, start, stop):
        nc.tensor.matmul(
            out,
            lhsT=lhsT.bitcast(f32r),
            rhs=rhs.bitcast(f32r),
            start=start,
            stop=stop,
        )

    def mmb(out, lhsT, rhs, start, stop):
        nc.tensor.matmul(out, lhsT=lhsT, rhs=rhs, start=start, stop=stop)

    C = nc.dram_tensor("C", [BB, H, LC], f32, kind="ExternalInput")
    Q = nc.dram_tensor("Q", [BB, H, LQ], f32, kind="ExternalInput")
    w = nc.dram_tensor("w", [3 * H], f32, kind="ExternalInput")
    W_res = nc.dram_tensor("W_res", [H, 4 * H], f32, kind="ExternalInput")
    b_res = nc.dram_tensor("b_res", [H], f32, kind="ExternalInput")
    out = nc.dram_tensor("out", [BB, H, LC], f32, kind="ExternalOutput")

    KH = H // 128  # 2 h-chunks
    NLT = LC // 128  # 8 l-tiles
    NMT = LQ // 128  # 2 m-tiles

    with tile.TileContext(nc) as tc:
        with ExitStack() as ctx:
            singles = ctx.enter_context(tc.tile_pool(name="singles", bufs=1))
            sb = ctx.enter_context(tc.tile_pool(name="sb", bufs=2))
            sb1 = ctx.enter_context(tc.tile_pool(name="sb1", bufs=3))
            sbig = ctx.enter_context(tc.tile_pool(name="sbig", bufs=2))
            sbig1 = ctx.enter_context(tc.tile_pool(name="sbig1", bufs=3))
            ps_tr = ctx.enter_context(
                tc.tile_pool(name="ps_tr", bufs=2, space="PSUM")
            )
            ps_z = ctx.enter_context(
                tc.tile_pool(name="ps_z", bufs=2, space="PSUM")
            )
            ps_big = ctx.enter_context(
                tc.tile_pool(name="ps_big", bufs=2, space="PSUM")
            )

            # ---- one-time constants ----
            identity_bf = singles.tile([128, 128], bf16)
            make_identity(nc, identity_bf)


            # w = [w1|w2|w3] is contiguous in DRAM: one DMA loads all three
            # column sets ([128, 6]; element 128*i + p -> (p, i)), saving two
            # ~0.9us descriptor slots on the SP queue ahead of the C0 load.
            w_cols = singles.tile([128, 3 * KH], f32r)
            nc.sync.dma_start(
                out=w_cols,
                in_=w.ap()
                .rearrange("(i p) -> p i", i=3 * KH, p=128)
                .bitcast(f32r),
            )
            w1_col = w_cols[:, 0:KH]
            w2_col = w_cols[:, KH : 2 * KH]
            w3_col = w_cols[:, 2 * KH : 3 * KH].bitcast(f32)
            # W_res^T (bf16): WT_all[:, 256*f + 128*j : +128] = W_res[128j:128(j+1), 128f:128(f+1)]^T
            # Two loads only — early transfers compete with the batch-0 C/Q
            # loads for DMA hardware rings, so fewer is better.
            WT_all = singles.tile([128, 4 * H * KH], bf16)
            wn = {}
            for j in range(KH):
                t = singles.tile([128, 4 * H], f32, tag=f"wn{j}")
                nc.scalar.dma_start(out=t, in_=W_res.ap()[128 * j : 128 * (j + 1), :])
                tb = singles.tile([128, 4 * H], bf16, tag=f"wnb{j}")
                nc.vector.tensor_copy(tb, t)
                wn[j] = tb
            b_col = singles.tile([128, KH], f32)
            nc.scalar.dma_start(
                out=b_col, in_=b_res.ap().rearrange("(i p) -> p i", i=KH, p=128)
            )

            # sel8[k, 128*i + q] = (k == i): K=8 selector for broadcasting row i
            # of an [8, 128] tile to all 128 output partitions via one matmul.
            # Built with two affine_selects (keep where pred true, else fill 0)
            # bracketing the band 128p <= f < 128(p+1) — no DMAs, so the ACT
            # queue reaches batch-0's exps immediately.
            sel8 = singles.tile([NLT, NLT * 128], bf16)
            nc.gpsimd.memset(sel8, 1.0)
            nc.gpsimd.affine_select(
                out=sel8,
                in_=sel8,
                compare_op=OP.is_ge,
                fill=0.0,
                base=0,
                pattern=[[1, NLT * 128]],  # iota = f - 128*p
                channel_multiplier=-128,
            )
            nc.gpsimd.affine_select(
                out=sel8,
                in_=sel8,
                compare_op=OP.is_ge,
                fill=0.0,
                base=127,
                pattern=[[-1, NLT * 128]],  # iota = 127 + 128*p - f
                channel_multiplier=128,
            )

            seq = [(f, j) for f in range(8) for j in range(KH)]
            for g in range(4):
                pt = ps_tr.tile([128, 512], bf16, tag="tr")
                for s in range(4):
                    f, j = seq[4 * g + s]
                    nc.tensor.transpose(
                        pt[:, 128 * s : 128 * (s + 1)],
                        wn[j][:, 128 * f : 128 * (f + 1)],
                        identity_bf,
                    )
                nc.vector.tensor_copy(
                    out=WT_all[:, 512 * g : 512 * (g + 1)], in_=pt
                )

            def wt(f, t_i):
                return WT_all[:, 256 * f + 128 * t_i : 256 * f + 128 * (t_i + 1)]

            def loads(b):
                C_nat = []
                Q_nat = []
                for k in range(KH):
                    t = sbig.tile([128, LC], f32r, tag=f"cnat{k}", bufs=3)
                    nc.sync.dma_start(
                        out=t,
                        in_=C.ap()[b, 128 * k : 128 * (k + 1), :].bitcast(f32r),
                    )
                    C_nat.append(t)
                    tq = sb.tile([128, LQ], f32r, tag=f"qnat{k}", bufs=3)
                    nc.sync.dma_start(
                        out=tq,
                        in_=Q.ap()[b, 128 * k : 128 * (k + 1), :].bitcast(f32r),
                    )
                    Q_nat.append(tq)
                return C_nat, Q_nat

            def frontend(b, ld):
                st = {}
                C_nat, Q_nat = ld

                # ---- r_row = w1.C  [1, LC],  q_row = w2.Q  [1, LQ] (bf16) ----
                r_row = sb.tile([1, LC], bf16, tag="rrow")
                for c in range(2):
                    ps_r = ps_tr.tile([1, 512], f32, tag="tr")
                    for k in range(KH):
                        mm(
                            ps_r,
                            w1_col[:, k : k + 1],
                            C_nat[k][:, 512 * c : 512 * (c + 1)],
                            (k == 0),
                            (k == KH - 1),
                        )
                    nc.vector.tensor_copy(
                        out=r_row[:, 512 * c : 512 * (c + 1)], in_=ps_r
                    )
                ps_q = ps_tr.tile([1, LQ], f32, tag="tr")
                for k in range(KH):
                    mm(
                        ps_q,
                        w2_col[:, k : k + 1],
                        Q_nat[k],
                        (k == 0),
                        (k == KH - 1),
                    )
                q_row = sb.tile([1, LQ], bf16, tag="qrow")
                nc.vector.tensor_copy(out=q_row, in_=ps_q)

                # ---- flip r/q rows into per-partition bias columns via ten
                # ---- tiny N=1 matmuls (lhsT^T @ [1.0]), batched in one PSUM ----
                ptr_rq = ps_tr.tile([128, 16], f32, tag="tr")
                for i in range(NLT):
                    mmb(
                        ptr_rq[:, i : i + 1],
                        r_row[:, 128 * i : 128 * (i + 1)],
                        identity_bf[0:1, 0:1],
                        True,
                        True,
                    )
                for j in range(NMT):
                    mmb(
                        ptr_rq[:, NLT + j : NLT + j + 1],
                        q_row[:, 128 * j : 128 * (j + 1)],
                        identity_bf[0:1, 0:1],
                        True,
                        True,
                    )
                rq_col = sb.tile([128, NLT + NMT], f32, tag="rqcol")
                nc.vector.tensor_copy(rq_col, ptr_rq[:, 0 : NLT + NMT])

                # ---- bf16 copies ----
                C_bf = []
                Q_bf = []
                for k in range(KH):
                    cb = sbig.tile([128, LC], bf16, tag=f"cbf{k}")
                    nc.vector.tensor_copy(cb, C_nat[k].bitcast(f32))
                    C_bf.append(cb)
                    qb = sb.tile([128, LQ], bf16, tag=f"qbf{k}")
                    nc.vector.tensor_copy(qb, Q_nat[k].bitcast(f32))
                    Q_bf.append(qb)

                # ---- PE transposes, batched drains ----
                # CtT_all[:, 256*i + 128*k : +128] = C^T l-tile i, h-chunk k
                CtT_all = sb1.tile([128, 2 * H * NLT // 2], bf16, tag="ctt")
                cseq = [(i, k) for i in range(NLT) for k in range(KH)]
                for g in range(4):
                    pt = ps_tr.tile([128, 512], bf16, tag="tr")
                    for s in range(4):
                        i, k = cseq[4 * g + s]
                        nc.tensor.transpose(
                            pt[:, 128 * s : 128 * (s + 1)],
                            C_bf[k][:, 128 * i : 128 * (i + 1)],
                            identity_bf,
                        )
                    nc.vector.tensor_copy(
                        out=CtT_all[:, 512 * g : 512 * (g + 1)], in_=pt
                    )
                QT_all = sb.tile([128, H * NMT], bf16, tag="qt")
                qseq = [(j, k) for j in range(NMT) for k in range(KH)]
                pt = ps_tr.tile([128, 512], bf16, tag="tr")
                for s in range(4):
                    j, k = qseq[s]
                    nc.tensor.transpose(
                        pt[:, 128 * s : 128 * (s + 1)],
                        Q_bf[k][:, 128 * j : 128 * (j + 1)],
                        identity_bf,
                    )
                nc.vector.tensor_copy(out=QT_all, in_=pt)

                # ---- affine-augmented operands (GpSimd: SBUF-only) ----
                # CA = C*w3 + w2 so CA^T@Q = dot + q[m]; QA = Q*w3 + w1 so
                # QA^T@C = dot + r[l]. bf16: the PE runs bf16 at 1 cyc/col
                # vs ~1.1-1.4 for fp32r, and accumulation stays fp32.
                CA = []
                QA = []
                for k in range(KH):
                    t = sbig.tile([128, LC], bf16, tag=f"ca{k}")
                    eng = nc.gpsimd if k == 0 else nc.vector
                    eng.tensor_scalar(
                        out=t,
                        in0=C_nat[k],
                        scalar1=w3_col[:, k : k + 1],
                        scalar2=w2_col[:, k : k + 1].bitcast(f32),
                        op0=OP.mult,
                        op1=OP.add,
                    )
                    CA.append(t)
                    tq = sb.tile([128, LQ], bf16, tag=f"qa{k}")
                    nc.vector.tensor_scalar(
                        out=tq,
                        in0=Q_nat[k],
                        scalar1=w3_col[:, k : k + 1],
                        scalar2=w1_col[:, k : k + 1].bitcast(f32),
                        op0=OP.mult,
                        op1=OP.add,
                    )
                    QA.append(tq)

                st.update(
                    C_nat=C_nat, Q_nat=Q_nat, C_bf=C_bf, Q_bf=Q_bf,
                    CtT_all=CtT_all, QT_all=QT_all, CA=CA, QA=QA,
                    rq_col=rq_col,
                )
                return st

            def backend(b, st):
                C_nat = st["C_nat"]; Q_nat = st["Q_nat"]; C_bf = st["C_bf"]
                Q_bf = st["Q_bf"]; CtT_all = st["CtT_all"]; QT_all = st["QT_all"]
                CA = st["CA"]; QA = st["QA"]; rq_col = st["rq_col"]

                # ---- Z in [l, m] layout + rowsums rho ----
                # S = (C*w3)^T Q + r x 1 + 1 x q ; the rank-1 terms enter the
                # PSUM accumulation directly (fp32r), no exp-bias needed.
                # Two l-tiles share each [128, 512] PSUM tile (the ring
                # slots are 2KB anyway): separate accumulation groups and
                # exps per half, but the 2-slot ring now gives each matmul
                # group four exps of slack instead of two.
                rho_col = sb.tile([128, NLT], f32, tag="rho")
                E_lm = []
                for p in range(NLT // 2):
                    pz = ps_z.tile([128, 2 * LQ], f32, tag="z")
                    for h in range(2):
                        i = 2 * p + h
                        sl = slice(LQ * h, LQ * (h + 1))
                        for k in range(KH):
                            mmb(
                                pz[:, sl],
                                CA[k][:, 128 * i : 128 * (i + 1)],
                                Q_bf[k],
                                (k == 0),
                                (k == KH - 1),
                            )
                        e = sb1.tile([128, LQ], bf16, tag=f"elm{i}", name=f"elm{i}")
                        nc.scalar.activation(
                            out=e,
                            in_=pz[:, sl],
                            func=AF.Exp,
                            bias=rq_col[:, i : i + 1],
                            accum_out=rho_col[:, i : i + 1],
                        )
                        E_lm.append(e)

                # ---- Z in [m, l] layout + colsums kappa ----
                kap_col = sb.tile([128, NMT], f32, tag="kap")
                E_ml = []
                for j in range(NMT):
                    pzt = ps_big.tile([128, LC], f32, tag="big")
                    for c in range(2):
                        sl = slice(512 * c, 512 * (c + 1))
                        for k in range(KH):
                            mmb(
                                pzt[:, sl],
                                QA[k][:, 128 * j : 128 * (j + 1)],
                                C_bf[k][:, sl],
                                (k == 0),
                                (k == KH - 1),
                            )
                    e = sbig1.tile([128, LC], bf16, tag=f"eml{j}")
                    nc.scalar.activation(
                        out=e,
                        in_=pzt,
                        func=AF.Exp,
                        bias=rq_col[:, NLT + j : NLT + j + 1],
                        accum_out=kap_col[:, j : j + 1],
                    )
                    E_ml.append(e)

                # ---- reciprocals ----
                kap_inv = sb.tile([128, NMT], f32, tag="kapi")
                nc.vector.reciprocal(kap_inv, kap_col)

                # ---- T = S_col^T @ Ct   [m, h] ----
                T_nat = []
                for j in range(NMT):
                    pT = ps_z.tile([128, H], f32, tag="z")
                    for i in range(NLT):
                        mmb(
                            pT,
                            E_lm[i][:, 128 * j : 128 * (j + 1)],
                            CtT_all[:, 256 * i : 256 * (i + 1)],
                            (i == 0),
                            (i == NLT - 1),
                        )
                    t = sb1.tile([128, H], bf16, tag=f"tn{j}")
                    nc.scalar.activation(
                        out=t, in_=pT, func=AF.Copy, scale=kap_inv[:, j : j + 1]
                    )
                    T_nat.append(t)

                # ---- 1/rho broadcast: column -> rows -> [128, LC] ----
                rho_inv = sb.tile([128, NLT], f32, tag="rhoi")
                nc.vector.reciprocal(rho_inv, rho_col)
                rho_inv_bf = sb.tile([128, NLT], bf16, tag="rhoib")
                nc.gpsimd.tensor_copy(rho_inv_bf, rho_inv)
                ptr = ps_tr.tile([NLT, 128], bf16, tag="tr")
                nc.tensor.transpose(ptr, rho_inv_bf, identity_bf)
                rho_rows = sb.tile([NLT, 128], bf16, tag="rrows")
                nc.vector.tensor_copy(rho_rows, ptr)
                ri_bc = sbig1.tile([128, LC], bf16, tag="ribc")
                for half in range(2):
                    pri = ps_tr.tile([128, 512], f32, tag="tr")
                    for s in range(4):
                        i = 4 * half + s
                        mmb(
                            pri[:, 128 * s : 128 * (s + 1)],
                            sel8[:, 128 * i : 128 * (i + 1)],
                            rho_rows,
                            True,
                            True,
                        )
                    nc.vector.tensor_copy(
                        out=ri_bc[:, 512 * half : 512 * (half + 1)], in_=pri
                    )

                # ---- A^T and Bv^T  [h, l]: matmuls on unscaled E_ml, the
                # ---- PSUM drain multiplies in 1/rho[l] ----
                A_T = []
                Bv_T = []
                for t_i in range(KH):
                    pA = ps_big.tile([128, LC], f32, tag="big")
                    for k in range(NMT):
                        for c in range(2):
                            sl = slice(512 * c, 512 * (c + 1))
                            mmb(
                                pA[:, sl],
                                QT_all[:, 256 * k + 128 * t_i : 256 * k + 128 * (t_i + 1)],
                                E_ml[k][:, sl],
                                (k == 0),
                                (k == NMT - 1),
                            )
                    a = sbig1.tile([128, LC], bf16, tag=f"at{t_i}")
                    nc.vector.tensor_mul(a, pA, ri_bc)
                    A_T.append(a)
                for t_i in range(KH):
                    pB = ps_big.tile([128, LC], f32, tag="big")
                    for k in range(NMT):
                        for c in range(2):
                            sl = slice(512 * c, 512 * (c + 1))
                            mmb(
                                pB[:, sl],
                                T_nat[k][:, 128 * t_i : 128 * (t_i + 1)],
                                E_ml[k][:, sl],
                                (k == 0),
                                (k == NMT - 1),
                            )
                    bv = sbig1.tile([128, LC], bf16, tag=f"bvt{t_i}")
                    nc.vector.tensor_mul(bv, pB, ri_bc)
                    Bv_T.append(bv)

                # ---- products ----
                CA1 = []
                CB1 = []
                for t_i in range(KH):
                    p1 = sbig1.tile([128, LC], bf16, tag=f"ca1{t_i}")
                    nc.vector.tensor_tensor(
                        out=p1, in0=C_bf[t_i], in1=A_T[t_i], op=OP.mult
                    )
                    CA1.append(p1)
                    p2 = sbig1.tile([128, LC], bf16, tag=f"cb1{t_i}")
                    nc.vector.tensor_tensor(
                        out=p2, in0=C_bf[t_i], in1=Bv_T[t_i], op=OP.mult
                    )
                    CB1.append(p2)

                # ---- final matmul + relu + store ----
                blocks = [
                    C_bf[0],
                    C_bf[1],
                    A_T[0],
                    A_T[1],
                    CA1[0],
                    CA1[1],
                    CB1[0],
                    CB1[1],
                ]
                # c-outer so each 512-chunk drains (relu + store) while the
                # PE accumulates the other chunk.
                for t_i in range(KH):
                    po = ps_big.tile([128, LC], f32, tag="big")
                    for c in range(2):
                        sl = slice(512 * c, 512 * (c + 1))
                        for f in range(8):
                            mmb(
                                po[:, sl],
                                wt(f, t_i),
                                blocks[f][:, sl],
                                (f == 0),
                                (f == 7),
                            )
                        o = sbig.tile([128, 512], f32, tag=f"osb{t_i}{c}")
                        nc.scalar.activation(
                            out=o,
                            in_=po[:, sl],
                            func=AF.Relu,
                            bias=b_col[:, t_i : t_i + 1],
                        )
                        nc.sync.dma_start(
                            out=out.ap()[b, 128 * t_i : 128 * (t_i + 1), sl],
                            in_=o,
                        )

            def body(iv=None):
                ld = {0: loads(0), 1: loads(1)}
                st_prev = None
                for b in range(BB):
                    if b + 2 < BB:
                        ld[b + 2] = loads(b + 2)
                    st = frontend(b, ld.pop(b))
                    if st_prev is not None:
                        backend(b - 1, st_prev)
                    st_prev = st
                backend(BB - 1, st_prev)

            if reps == 1:
                body()
            else:
                with tc.For_i(0, reps, 1) as iv:
                    body(iv)

    nc.compile()
    return nc


def _get_nc(reps: int = 1):
    key = ("nc", reps)
    if key not in _CACHE:
        _CACHE[key] = _build(reps)
    return _CACHE[key]


def kernel(C, Q, cmask, qmask, w, W_res, b_res, _reps: int = 1, _want_res: bool = False,
           _trace: bool = False, _tmpdir: str | None = None):
    from concourse.bass_utils import run_bass_kernel_spmd

    nc = _get_nc(_reps)

    C = np.ascontiguousarray(C, dtype=np.float32)
    Q = np.ascontiguousarray(Q, dtype=np.float32)
    w = np.ascontiguousarray(w, dtype=np.float32)
    W_res = np.ascontiguousarray(W_res, dtype=np.float32)
    b_res = np.ascontiguousarray(b_res, dtype=np.float32)

    in_maps = []
    for i in range(N_CORES):
        sl = slice(i * BB, (i + 1) * BB)
        in_maps.append(
            {"C": C[sl], "Q": Q[sl], "w": w, "W_res": W_res, "b_res": b_res}
        )

    res = run_bass_kernel_spmd(
        nc, in_maps, core_ids=list(range(N_CORES)), trace=_trace, tmpdir=_tmpdir
    )
    out = np.concatenate([res.results[i]["out"] for i in range(N_CORES)], axis=0)
    if _want_res:
        return out, res
    return out
